# revision 28
# baseline (speedup 1.0000x reference)
"""BiDAF attention layer on 8 Trainium2 NeuronCores (Bass/Tile).

Math (per batch b):
  t[i,j]  = sum_d (c[i,d]*w_cq[d] + w_q[d]) * q[j,d]   (= cq + sq0[j])
  a       = softmax_j(t)            (biases b_c/b_q/b_cq cancel in softmax)
  c2q     = a @ q
  m[i]    = max_j t[i,j];  sc0[i] = c[i,:]@w_c
  bvec    = softmax_i(m + sc0)      (biases cancel here too)
  q2c     = bvec @ c
  out     = [c | c2q | c*c2q | c*q2c]

Sharding: data-parallel over batch, 4 batches per core, params replicated.

Design notes:
  - Scores are computed ONCE, in [j,i] layout only.  The per-i row max is
    recovered from e = exp(t) itself: since bvec numerators are
    exp(m + sc0 - shift) = (max_j e) * exp(sc0 - shift), no log is needed.
    max_j e is a partition-axis max of eT: free-dim max over the NJ chunks
    (DVE) -> PE transposes of the [j,128] rows -> free-dim reduce_max.
  - c is loaded once (f32) straight into block 0 of the output staging
    rows; the fp16 copy for the PE is cast on-chip (no second HBM read).
  - sc0 = c @ w_c per-column via fused DVE scalar_tensor_tensor accum_out.
  - Blocks 0..2 of each output row are staged in SBUF and stored with one
    3 KiB-descriptor DMA per half-batch as soon as c2q for that half is
    done (no q2c dependency); block 3 (c*q2c) is stored separately per
    half on the sync ring.
  - PE order keeps the array warm (HAM) through the bvec latency chain:
    scores(b) -> prep transposes(b+1) -> c2q h0(b) -> emax transposes(b)
    -> q2c(b) -> c2q h1(b) -> scores(b+1).
  - chatT = w_cq*cT + w_q evacuation runs on ACT (activation Identity
    with per-partition scale+bias); block1 = po/l on ACT; block2 =
    (po/l)*c fused on DVE; c4 = c*q2c on GpSimd (DVE for the last batch).
  - Batch 0's q comes over the sync ring as f32 + DVE cast (SWDGE
    descriptor generation would delay it to ~10us); later batches use
    SWDGE cast-in-flight loads.  Params ride the scalar ring, packed into
    two DMAs, so c(b0) heads the sync ring.
"""

import sys

if "/opt/trn_rl_repo" not in sys.path:
    sys.path.insert(0, "/opt/trn_rl_repo")

import numpy as np

import concourse.bass as bass
import concourse.tile as tile
from concourse import bacc, mybir
from concourse.bass import ds, ts

B, CL, QL, D = 32, 1024, 512, 256
NCORES = 8
BS = B // NCORES  # batches per core
P = 128
F32 = mybir.dt.float32
F16 = mybir.dt.float16

NT = CL // P  # 8 i-tiles
NJ = QL // P  # 4 j-chunks
ND = D // P   # 2 d-chunks
NH = 2        # i-halves
IH = CL // NH  # 512
KPH = NT // NH  # 4 i-tiles per half

Exp = mybir.ActivationFunctionType.Exp
Ident = mybir.ActivationFunctionType.Identity
AxX = mybir.AxisListType.X
Mult = mybir.AluOpType.mult
Add = mybir.AluOpType.add

SHIFT = -2.5  # bvec numerator shift, keeps exp() in fp16-friendly range


def build_bass(bs: int = BS):
    nc = bacc.Bacc(None)
    c_d = nc.declare_dram_parameter("c", [bs, CL, D], F32, isOutput=False)
    q_d = nc.declare_dram_parameter("q", [bs, QL, D], F32, isOutput=False)
    # params packed host-side: fp16 [ident | wc_bcast], f32 [ones | wq | wcq]
    ph_d = nc.declare_dram_parameter("params_h", [P, P + D], F16, isOutput=False)
    pf_d = nc.declare_dram_parameter(
        "params_f", [P, P + 2 * ND + 1], F32, isOutput=False
    )
    out_d = nc.declare_dram_parameter("out", [bs, CL, 4 * D], F32, isOutput=True)

    with tile.TileContext(nc) as tc:
        with (
            tc.tile_pool(name="consts", bufs=1) as consts,
            tc.tile_pool(name="stage", bufs=3) as stage_pool,
            tc.tile_pool(name="c4p", bufs=2) as c4p,
            tc.tile_pool(name="qin", bufs=3) as qin,
            tc.tile_pool(name="ch", bufs=2) as chp,
            tc.tile_pool(name="work", bufs=2) as work,
            tc.tile_pool(name="small", bufs=3) as small,
            tc.tile_pool(name="ps_sc", bufs=3, space="PSUM") as ps_sc,
            tc.tile_pool(name="ps_sm", bufs=2, space="PSUM") as ps_sm,
            tc.tile_pool(name="ps_po", bufs=3, space="PSUM") as ps_po,
        ):
            par_h = consts.tile([P, P + D], F16)
            par_f = consts.tile([P, P + 2 * ND + 1], F32)
            ident_h = par_h[:, 0:P]
            wcb_sb = par_h[:, P : P + D]
            ones_f = par_f[:, 0:P]
            wq_sb = par_f[:, P : P + ND]
            wcq_sb = par_f[:, P + ND : P + 2 * ND]
            neg_shift = par_f[:, P + 2 * ND : P + 2 * ND + 1]

            def emit_inputs(b):
                q_sb = qin.tile([P, NJ, D + 1], F16, tag="q_sb")
                if b == 0:
                    # sync-ring f32 load (ordered FIRST) + DVE cast: ready
                    # ~8us before the SWDGE path could deliver it
                    q_f = qin.tile([P, NJ, D], F32, tag="q_f")
                    nc.sync.dma_start(
                        out=q_f, in_=q_d[b].rearrange("(t p) d -> p t d", p=P)
                    )
                    nc.vector.tensor_copy(q_sb[:, :, 0:D], q_f)
                else:
                    nc.gpsimd.dma_start(
                        out=q_sb[:, :, 0:D],
                        in_=q_d[b].rearrange("(t p) d -> p t d", p=P),
                    )
                nc.vector.memset(q_sb[:, :, D : D + 1], 1.0)
                stg = []
                for h in range(NH):
                    st = stage_pool.tile([P, KPH, 3 * D], F32, tag=f"stage{h}")
                    cv = c_d[b].rearrange("(t p) d -> p t d", p=P)
                    nc.sync.dma_start(
                        out=st[:, :, 0:D], in_=cv[:, ds(h * KPH, KPH)]
                    )
                    stg.append(st)
                # fp16 copy of c, cast split across ACT / DVE
                c_h = chp.tile([P, NT, D], F16, tag="c_h")
                nc.scalar.copy(c_h[:, 0:KPH], stg[0][:, :, 0:D])
                nc.vector.tensor_copy(c_h[:, KPH:NT], stg[1][:, :, 0:D])
                ov = out_d[b].rearrange("(t p) x -> p t x", p=P)
                return q_sb, stg, c_h, ov

            def emit_prep(b, q_sb, c_h):
                """transposes + evacs for batch b (PE-heavy part)."""
                qT = work.tile([P, ND, QL], F16, tag="qT")
                for dc in range(ND):
                    pst = ps_sm.tile([P, QL], F16, tag="t")
                    for jc in range(NJ):
                        nc.tensor.transpose(
                            pst[:, ts(jc, P)], q_sb[:, jc, ts(dc, P)], ident_h
                        )
                    nc.vector.tensor_copy(qT[:, dc], pst)
                chatT = work.tile([P, ND, CL], F16, tag="chatT")
                for dc in range(ND):
                    for h in range(NH):
                        pst = ps_sm.tile([P, IH], F16, tag="t")
                        for k in range(KPH):
                            it = h * KPH + k
                            nc.tensor.transpose(
                                pst[:, ts(k, P)], c_h[:, it, ts(dc, P)], ident_h
                            )
                        nc.scalar.activation(
                            chatT[:, dc, ds(h * IH, IH)],
                            pst,
                            Ident,
                            bias=wq_sb[:, dc : dc + 1],
                            scale=wcq_sb[:, dc : dc + 1],
                        )
                return c_h, qT, chatT

            def emit_sc0(b, prep):
                """sc0 = c @ w_c (fused STT, split DVE/GpSimd); off hot path."""
                c_h, qT, chatT = prep
                sc0_col = small.tile([P, NT], F32, tag="sc0")
                junk = small.tile([P, D], F16, tag="junk")
                for it in range(NT):
                    nc.vector.scalar_tensor_tensor(
                        out=junk,
                        in0=c_h[:, it],
                        scalar=1.0,
                        in1=wcb_sb,
                        op0=Mult,
                        op1=Mult,
                        accum_out=sc0_col[:, it : it + 1],
                    )
                e_sc0 = small.tile([P, NT], F32, tag="esc0")
                nc.scalar.activation(e_sc0, sc0_col, Exp, bias=neg_shift)
                return e_sc0

            def emit_scores(b, prep):
                """tT scores + exp + NJ-chunk max; returns eT, emax halves."""
                c_h, qT, chatT = prep
                eT = work.tile([P, NJ, CL], F16, tag="eT")
                emaxs = []
                for h in range(NH):
                    hsl = ds(h * IH, IH)
                    for jc in range(NJ):
                        pss = ps_sc.tile([P, IH], F32, tag="s")
                        for dc in range(ND):
                            nc.tensor.matmul(
                                pss,
                                qT[:, dc, ts(jc, P)],
                                chatT[:, dc, hsl],
                                start=(dc == 0),
                                stop=(dc == ND - 1),
                            )
                        nc.scalar.activation(eT[:, jc, hsl], pss, Exp)
                    ma = small.tile([P, IH], F16, tag="ma")
                    mb = small.tile([P, IH], F16, tag="mb")
                    nc.vector.tensor_max(ma, eT[:, 0, hsl], eT[:, 1, hsl])
                    nc.vector.tensor_max(mb, eT[:, 2, hsl], eT[:, 3, hsl])
                    emax_h = small.tile([P, IH], F16, tag="emaxh")
                    nc.vector.tensor_max(emax_h, ma, mb)
                    emaxs.append(emax_h)
                return eT, emaxs

            def emit_bvec(b, e_sc0, emaxs):
                """partition-max via PE transposes + ebv/colsum (DVE)."""
                emax_col = small.tile([P, NT], F32, tag="emaxc")
                for h in range(NH):
                    pst = ps_sm.tile([P, KPH, P], F16, tag="t")
                    for k in range(KPH):
                        nc.tensor.transpose(
                            pst[:, k], emaxs[h][:, ts(k, P)], ident_h
                        )
                    nc.vector.reduce_max(
                        emax_col[:, ds(h * KPH, KPH)], pst, AxX
                    )
                ebv_f = small.tile([P, NT], F32, tag="ebvf")
                colsum = small.tile([P, 1], F32, tag="colsum")
                nc.vector.scalar_tensor_tensor(
                    out=ebv_f,
                    in0=emax_col,
                    scalar=1.0,
                    in1=e_sc0,
                    op0=Mult,
                    op1=Mult,
                    accum_out=colsum,
                )
                ebv_h = small.tile([P, NT], F16, tag="ebvh")
                nc.vector.tensor_copy(ebv_h, ebv_f)
                return ebv_h, colsum

            def emit_q2c(b, prep, ebv_h, colsum):
                """q2c = (ebv @ c) / total, broadcast to all partitions."""
                c_h, qT, chatT = prep
                ps_tot = ps_sm.tile([P, 1], F32, tag="t")
                nc.tensor.matmul(ps_tot, ones_f, colsum, start=True, stop=True)
                totinv = small.tile([P, 1], F32, tag="totinv")
                nc.vector.reciprocal(totinv, ps_tot)
                ps_q2c = ps_sm.tile([1, D], F32, tag="t")
                for it in range(NT):
                    nc.tensor.matmul(
                        ps_q2c,
                        ebv_h[:, it : it + 1],
                        c_h[:, it],
                        start=(it == 0),
                        stop=(it == NT - 1),
                    )
                q2c_row = small.tile([1, D], F32, tag="q2cr")
                nc.vector.tensor_scalar_mul(q2c_row, ps_q2c, totinv[0:1, 0:1])
                ps_q2cb = ps_sm.tile([P, D], F32, tag="t")
                nc.tensor.matmul(
                    ps_q2cb, ones_f[0:1, :], q2c_row, start=True, stop=True
                )
                q2c_sb = small.tile([P, D], F32, tag="q2csb")
                nc.scalar.copy(q2c_sb, ps_q2cb)
                return q2c_sb

            def emit_c2q(b, h, eT, q_sb, stg, ov, last):
                """c2q matmuls + blocks 1+2 evac + store blocks 0..2.

                MM groups and evacs interleave so at most 3 po tiles are
                live (pool bufs=3) and the PE never waits on an evac that
                has not been emitted yet."""
                st = stg[h]
                pos = []

                def mm(k):
                    it = h * KPH + k
                    po = ps_po.tile([P, D + 1], F32, tag="po")
                    for jc in range(NJ):
                        nc.tensor.matmul(
                            po,
                            eT[:, jc, ts(it, P)],
                            q_sb[:, jc],
                            start=(jc == 0),
                            stop=(jc == NJ - 1),
                        )
                    pos.append(po)

                def evac(k):
                    po = pos[k]
                    linv = small.tile([P, 1], F32, tag="linv")
                    nc.vector.reciprocal(linv, po[:, D : D + 1])
                    nc.scalar.mul(st[:, k, D : 2 * D], po[:, 0:D], linv)
                    nc.vector.scalar_tensor_tensor(
                        out=st[:, k, 2 * D : 3 * D],
                        in0=po[:, 0:D],
                        scalar=linv,
                        in1=st[:, k, 0:D],
                        op0=Mult,
                        op1=Mult,
                    )

                mm(0)
                mm(1)
                evac(0)
                mm(2)
                evac(1)
                if last:
                    # finer store granularity so the tail drains sooner
                    nc.scalar.dma_start(
                        out=ov[:, ds(h * KPH, 2), 0 : 3 * D], in_=st[:, 0:2]
                    )
                mm(3)
                evac(2)
                evac(3)
                if last:
                    nc.scalar.dma_start(
                        out=ov[:, ds(h * KPH + 2, 2), 0 : 3 * D],
                        in_=st[:, 2:KPH],
                    )
                else:
                    nc.scalar.dma_start(
                        out=ov[:, ds(h * KPH, KPH), 0 : 3 * D], in_=st
                    )

            def emit_c4(b, h, stg, q2c_sb, ov, last):
                """block3 = c * q2c for half h + store on the sync ring."""
                c4t = c4p.tile([P, KPH, D], F32, tag=f"c4_{h}")
                eng = nc.vector if last else nc.gpsimd
                for k in range(KPH):
                    eng.tensor_mul(c4t[:, k], stg[h][:, k, 0:D], q2c_sb)
                nc.sync.dma_start(
                    out=ov[:, ds(h * KPH, KPH), 3 * D : 4 * D], in_=c4t
                )

            # ---------------- main schedule ----------------
            nc.scalar.dma_start(out=par_h, in_=ph_d[:])
            nc.scalar.dma_start(out=par_f, in_=pf_d[:])

            pending = [emit_inputs(0)]
            if bs > 1:
                pending.append(emit_inputs(1))
            preps = [emit_prep(0, pending[0][0], pending[0][2])]
            esc0s = [emit_sc0(0, preps[0])]

            for b in range(bs):
                q_sb, stg, c_h, ov = pending.pop(0)
                prep = preps.pop(0)
                e_sc0 = esc0s.pop(0)
                last = b == bs - 1
                if b + 2 < bs:
                    pending.append(emit_inputs(b + 2))

                eT, emaxs = emit_scores(b, prep)

                # PE keeps running: next batch's transposes cover the
                # exp/NJ-max latency, then emax transposes, then c2q
                if not last:
                    nq, _, nch, _ = pending[0]
                    preps.append(emit_prep(b + 1, nq, nch))

                ebv_h, colsum = emit_bvec(b, e_sc0, emaxs)
                emit_c2q(b, 0, eT, q_sb, stg, ov, last)
                q2c_sb = emit_q2c(b, prep, ebv_h, colsum)
                emit_c4(b, 0, stg, q2c_sb, ov, last)

                if not last:
                    esc0s.append(emit_sc0(b + 1, preps[0]))

                emit_c2q(b, 1, eT, q_sb, stg, ov, last)
                emit_c4(b, 1, stg, q2c_sb, ov, last)

    nc.compile()
    return nc


_NC_CACHE = {}


def _get_nc(bs: int = BS):
    if bs not in _NC_CACHE:
        _NC_CACHE[bs] = build_bass(bs)
    return _NC_CACHE[bs]


def _param_maps(w_c, w_q, w_cq):
    wc = np.asarray(w_c, np.float32)
    params_h = np.concatenate(
        [np.eye(P, dtype=np.float16),
         np.broadcast_to(wc.astype(np.float16), (P, D))],
        axis=1,
    )
    wq_cols = np.asarray(w_q, np.float32).reshape(ND, P).T
    wcq_cols = np.asarray(w_cq, np.float32).reshape(ND, P).T
    params_f = np.concatenate(
        [np.ones((P, P), np.float32), wq_cols, wcq_cols,
         np.full((P, 1), SHIFT, np.float32)],
        axis=1,
    )
    return np.ascontiguousarray(params_h), np.ascontiguousarray(params_f)


def _run(c, q, w_c, w_q, w_cq, trace=False, **trace_kwargs):
    from concourse.bass_utils import run_bass_kernel_spmd

    c = np.asarray(c, np.float32)
    q = np.asarray(q, np.float32)
    params_h, params_f = _param_maps(w_c, w_q, w_cq)

    nc = _get_nc(BS)
    in_maps = []
    for k in range(NCORES):
        in_maps.append(
            {
                "c": np.ascontiguousarray(c[k * BS : (k + 1) * BS]),
                "q": np.ascontiguousarray(q[k * BS : (k + 1) * BS]),
                "params_h": params_h,
                "params_f": params_f,
            }
        )
    res = None
    last_err = None
    for attempt in range(3):
        try:
            res = run_bass_kernel_spmd(
                nc,
                in_maps,
                core_ids=list(range(NCORES)),
                trace=trace,
                **trace_kwargs,
            )
            break
        except Exception as e:  # transient device wedges clear on retry
            last_err = e
            if "UNRECOVERABLE" not in str(e) and "UNAVAILABLE" not in str(e):
                raise
    if res is None:
        raise last_err
    out = np.concatenate([res.results[k]["out"] for k in range(NCORES)], axis=0)
    return out, res


def kernel(c, q, w_c, b_c, w_q, b_q, w_cq, b_cq):
    # b_c/b_q/b_cq provably cancel in both softmaxes; output doesn't use them.
    out, _ = _run(c, q, w_c, w_q, w_cq)
    return out
